# revision 10
# baseline (speedup 1.0000x reference)
"""Trainium2 Bass kernel for nn_AttentionNetwork (ragged path attention).

Data-parallel over 8 NeuronCores: 512 paths per core. Per core, stage 1
processes 64 blocks of 8 paths: node-MLP (bf16 matmuls, h^T layout),
length-masked softmax over nodes (mask folded into the score matmul as a
K=1 accumulate; exp broadcast across partitions via a K=1 outer-product
matmul), then the softmax-weighted node sum on the vector engine.
Stage 2 (fp32) computes path-attention scores and returns exp-weighted
partial sums + (max, sumexp) stats; the host combines the 8 partials.
"""

import sys

if "/opt/trn_rl_repo" not in sys.path:
    sys.path.insert(0, "/opt/trn_rl_repo")

from contextlib import ExitStack

import ml_dtypes
import numpy as np

import concourse.bass as bass  # noqa: F401  (import keeps bass registered)
import concourse.mybir as mybir
import concourse.tile as tile
from concourse import bacc, bass_utils

P, LMAX, D, H = 4096, 64, 512, 512
NCORES = 8
PS = P // NCORES          # paths per core
BP = 8                    # paths per block
NB = PS // BP             # blocks per core
R = BP * LMAX             # rows per block
KC = D // 128             # contraction chunks
HC = H // 128             # hidden tiles
MASK_NEG = -30000.0

f32 = mybir.dt.float32
bf16 = mybir.dt.bfloat16
AF = mybir.ActivationFunctionType
ALU = mybir.AluOpType
AX = mybir.AxisListType

LAST_RESULT = None
_PROG = None
_TRACE_KW = {}


def _build_program(nb=NB, stage="full"):
    nc = bacc.Bacc("TRN2", target_bir_lowering=False, debug=False, num_devices=NCORES)

    xt = nc.dram_tensor("xt", [nb, KC, 128, R], bf16, kind="ExternalInput")
    msk = nc.dram_tensor("msk", [nb, R], bf16, kind="ExternalInput")
    w1 = nc.dram_tensor("w1", [KC, 128, H], bf16, kind="ExternalInput")
    w2 = nc.dram_tensor("w2", [128, HC], bf16, kind="ExternalInput")
    b1 = nc.dram_tensor("b1", [128, HC], f32, kind="ExternalInput")
    aw1 = nc.dram_tensor("aw1", [KC, 128, H], f32, kind="ExternalInput")
    ab1 = nc.dram_tensor("ab1", [128, HC], f32, kind="ExternalInput")
    aw2 = nc.dram_tensor("aw2", [128, HC], f32, kind="ExternalInput")
    one1_bf = nc.dram_tensor("one1_bf", [1, 1], bf16, kind="ExternalInput")
    ones_bf = nc.dram_tensor("ones_bf", [1, 128], bf16, kind="ExternalInput")
    ones_f32 = nc.dram_tensor("ones_f32", [1, 128], f32, kind="ExternalInput")
    out_part = nc.dram_tensor("out_part", [128, KC], f32, kind="ExternalOutput")
    out_stats = nc.dram_tensor("out_stats", [1, 2], f32, kind="ExternalOutput")
    ps_here = nb * BP
    dbg = None
    if stage != "full":
        dbg = nc.dram_tensor("dbg", [128, KC, ps_here], f32, kind="ExternalOutput")

    with ExitStack() as ctx:
        tc = ctx.enter_context(tile.TileContext(nc))
        const = ctx.enter_context(tc.tile_pool(name="const", bufs=1))
        xpool = ctx.enter_context(tc.tile_pool(name="x", bufs=3))
        hpool = ctx.enter_context(tc.tile_pool(name="h", bufs=2))
        wpool = ctx.enter_context(tc.tile_pool(name="w", bufs=2))
        spool = ctx.enter_context(tc.tile_pool(name="s", bufs=3))
        ph_pool = ctx.enter_context(tc.tile_pool(name="ph", bufs=5, space="PSUM"))
        ps_pool = ctx.enter_context(tc.tile_pool(name="ps", bufs=2, space="PSUM"))
        pb_pool = ctx.enter_context(tc.tile_pool(name="pb", bufs=1, space="PSUM"))

        t_w1 = const.tile([128, KC, H], bf16)
        nc.sync.dma_start(t_w1[:], w1.ap().rearrange("k d h -> d k h"))
        t_w2 = const.tile([128, HC], bf16)
        nc.sync.dma_start(t_w2[:], w2.ap())
        t_b1 = const.tile([128, HC], f32)
        nc.sync.dma_start(t_b1[:], b1.ap())
        t_aw1 = const.tile([128, KC, H], f32)
        nc.sync.dma_start(t_aw1[:], aw1.ap().rearrange("k d h -> d k h"))
        t_ab1 = const.tile([128, HC], f32)
        nc.sync.dma_start(t_ab1[:], ab1.ap())
        t_aw2 = const.tile([128, HC], f32)
        nc.sync.dma_start(t_aw2[:], aw2.ap())
        t_one1 = const.tile([1, 1], bf16)
        nc.sync.dma_start(t_one1[:], one1_bf.ap())
        t_ones_bf = const.tile([1, 128], bf16)
        nc.sync.dma_start(t_ones_bf[:], ones_bf.ap())
        t_ones_f = const.tile([1, 128], f32)
        nc.sync.dma_start(t_ones_f[:], ones_f32.ap())

        pfT = const.tile([128, KC, ps_here], f32)  # normalized path features, d-major

        for b in range(nb):
            xts = []
            for k in range(KC):
                xk = xpool.tile([128, R], bf16, tag=f"x{k}")
                nc.sync.dma_start(xk[:], xt.ap()[b, k])
                xts.append(xk)
            mrow = spool.tile([1, R], bf16, tag="mrow")
            nc.sync.dma_start(mrow[:], msk.ap()[b : b + 1, :])

            rh_list = []
            for j in range(HC):
                ph = ph_pool.tile([128, R], f32, tag="h")
                for k in range(KC):
                    nc.tensor.matmul(
                        ph[:],
                        t_w1[:, k, 128 * j : 128 * (j + 1)],
                        xts[k][:],
                        start=(k == 0),
                        stop=(k == KC - 1),
                    )
                rh = hpool.tile([128, R], bf16, tag=f"rh{j}")
                nc.scalar.activation(rh[:], ph[:], AF.Relu, bias=t_b1[:, j : j + 1])
                rh_list.append(rh)

            # scores + additive length mask, accumulated in one PSUM group
            ps_s = ps_pool.tile([1, R], f32, tag="s")
            nc.tensor.matmul(
                ps_s[:], t_one1[:], mrow[:], start=True, stop=False,
                skip_group_check=True,
            )
            for j in range(HC):
                nc.tensor.matmul(
                    ps_s[:], t_w2[:, j : j + 1], rh_list[j][:],
                    start=False, stop=(j == HC - 1),
                    skip_group_check=True,
                )

            erow = spool.tile([1, R], bf16, tag="erow")
            nc.scalar.activation(erow[:], ps_s[:], AF.Exp)

            pw = pb_pool.tile([128, R], f32, tag="b")
            nc.tensor.matmul(pw[:], t_ones_bf[:], erow[:], start=True, stop=True)
            wful = wpool.tile([128, R], bf16, tag="wf")
            nc.scalar.copy(wful[:], pw[:])

            wsum = spool.tile([128, BP], f32, tag="wsum")
            nc.vector.reduce_sum(
                wsum[:], wful[:].rearrange("p (s l) -> p s l", l=LMAX), axis=AX.X
            )
            winv = spool.tile([128, BP], f32, tag="winv")
            nc.vector.reciprocal(winv[:], wsum[:])
            for k in range(KC):
                xw = xpool.tile([128, R], bf16, tag=f"xw{k}")
                nc.vector.tensor_mul(xw[:], xts[k][:], wful[:])
                praw = spool.tile([128, BP], f32, tag=f"praw{k}")
                nc.vector.reduce_sum(
                    praw[:], xw[:].rearrange("p (s l) -> p s l", l=LMAX), axis=AX.X
                )
                nc.vector.tensor_mul(
                    pfT[:, k, BP * b : BP * (b + 1)], praw[:], winv[:]
                )

        if stage == "wsum":
            nc.sync.dma_start(dbg.ap(), pfT[:])

        if stage == "full":
            # ---- stage 2: path-level attention (fp32) ----
            rh2_list = []
            for j in range(HC):
                ph2 = ph_pool.tile([128, ps_here], f32, tag="h")
                for k in range(KC):
                    nc.tensor.matmul(
                        ph2[:],
                        t_aw1[:, k, 128 * j : 128 * (j + 1)],
                        pfT[:, k, :],
                        start=(k == 0),
                        stop=(k == KC - 1),
                    )
                rh2 = hpool.tile([128, ps_here], f32, tag=f"rh2{j}")
                nc.scalar.activation(rh2[:], ph2[:], AF.Relu, bias=t_ab1[:, j : j + 1])
                rh2_list.append(rh2)

            ps_a = ps_pool.tile([1, ps_here], f32, tag="s")
            for j in range(HC):
                nc.tensor.matmul(
                    ps_a[:], t_aw2[:, j : j + 1], rh2_list[j][:],
                    start=(j == 0), stop=(j == HC - 1),
                )

            negm = spool.tile([1, 1], f32, tag="negm")
            nc.vector.reduce_max(negm[:], ps_a[:], axis=AX.X, negate=True)
            ea = spool.tile([1, ps_here], f32, tag="ea")
            s_t = spool.tile([1, 1], f32, tag="s1")
            nc.scalar.activation(ea[:], ps_a[:], AF.Exp, bias=negm[:], accum_out=s_t[:])

            pe_b = pb_pool.tile([128, ps_here], f32, tag="b")
            nc.tensor.matmul(pe_b[:], t_ones_f[:], ea[:], start=True, stop=True)
            ebc = wpool.tile([128, ps_here], f32, tag="ebc")
            nc.scalar.copy(ebc[:], pe_b[:])

            part = spool.tile([128, KC], f32, tag="part")
            for k in range(KC):
                scr = wpool.tile([128, ps_here], f32, tag="scr")
                nc.vector.tensor_mul(scr[:], pfT[:, k, :], ebc[:])
                nc.vector.reduce_sum(part[:, k : k + 1], scr[:], axis=AX.X)
            nc.sync.dma_start(out_part.ap(), part[:])
            nc.sync.dma_start(out_stats.ap()[:, 0:1], negm[:])
            nc.sync.dma_start(out_stats.ap()[:, 1:2], s_t[:])

    nc.compile()
    return nc


def _get_program():
    global _PROG
    if _PROG is None:
        _PROG = _build_program()
    return _PROG


def kernel(**inputs):
    global LAST_RESULT
    x = np.asarray(inputs["paths_nodes"], dtype=np.float32)
    lengths = np.asarray(inputs["lengths"], dtype=np.int32)
    pW1 = np.asarray(inputs["pW1"], dtype=np.float32)
    pb1 = np.asarray(inputs["pb1"], dtype=np.float32)
    pw2 = np.asarray(inputs["pw2"], dtype=np.float32)
    aW1 = np.asarray(inputs["aW1"], dtype=np.float32)
    ab1 = np.asarray(inputs["ab1"], dtype=np.float32)
    aw2 = np.asarray(inputs["aw2"], dtype=np.float32)
    # pb2 / ab2 shift their softmax logits uniformly -> no effect on output.

    nc = _get_program()

    bf = ml_dtypes.bfloat16
    # [P,L,D] -> per core [NB, KC, 128(d), R=BP*LMAX] with d on partitions
    xt_all = (
        x.reshape(NCORES, NB, BP, LMAX, KC, 128)
        .transpose(0, 1, 4, 5, 2, 3)
        .astype(bf)
        .reshape(NCORES, NB, KC, 128, R)
    )
    valid = np.arange(LMAX)[None, :] < lengths[:, None]
    msk_all = (
        np.where(valid, 0.0, MASK_NEG)
        .astype(bf)
        .reshape(NCORES, NB, R)
    )

    w1_np = np.ascontiguousarray(pW1.reshape(KC, 128, H)).astype(bf)
    w2_np = np.ascontiguousarray(pw2.reshape(HC, 128).T).astype(bf)
    b1_np = np.ascontiguousarray(pb1.reshape(HC, 128).T).astype(np.float32)
    aw1_np = np.ascontiguousarray(aW1.reshape(KC, 128, H)).astype(np.float32)
    ab1_np = np.ascontiguousarray(ab1.reshape(HC, 128).T).astype(np.float32)
    aw2_np = np.ascontiguousarray(aw2.reshape(HC, 128).T).astype(np.float32)
    one1 = np.ones((1, 1), dtype=bf)
    ones_b = np.ones((1, 128), dtype=bf)
    ones_f = np.ones((1, 128), dtype=np.float32)

    in_maps = []
    for c in range(NCORES):
        in_maps.append(
            {
                "xt": xt_all[c],
                "msk": msk_all[c],
                "w1": w1_np,
                "w2": w2_np,
                "b1": b1_np,
                "aw1": aw1_np,
                "ab1": ab1_np,
                "aw2": aw2_np,
                "one1_bf": one1,
                "ones_bf": ones_b,
                "ones_f32": ones_f,
            }
        )

    res = bass_utils.run_bass_kernel_spmd(
        nc, in_maps, core_ids=list(range(NCORES)), **_TRACE_KW
    )
    LAST_RESULT = res

    parts = np.stack([r["out_part"] for r in res.results])    # [8, 128, KC]
    stats = np.stack([r["out_stats"] for r in res.results])   # [8, 1, 2]
    m = -stats[:, 0, 0]
    s = stats[:, 0, 1]
    mg = m.max()
    sc = np.exp(m - mg)
    total = float((sc * s).sum())
    vec = (parts * sc[:, None, None]).sum(axis=0)             # [128, KC]
    user = np.ascontiguousarray(vec.T).reshape(D) / total
    return user.astype(np.float32)


# revision 14
# speedup vs baseline: 1.0536x; 1.0536x over previous
"""Trainium2 Bass kernel for nn_AttentionNetwork (ragged path attention).

Data-parallel over 8 NeuronCores: 512 paths per core. Per core, stage 1
processes 32 superblocks of 16 paths (1024 rows): node-MLP (bf16 matmuls,
h^T layout, N=1024 moving operand), length-masked softmax over nodes
(mask folded into the score-matmul PSUM group as a K=1 accumulate; exp
row broadcast across partitions on GpSimd), then the softmax-weighted
node sum on the vector engine with merged broadcast-AP ops.
Stage 2 (fp32) computes path-attention scores and returns exp-weighted
partial sums + (max, sumexp) stats; the host combines the 8 partials.
"""

import sys

if "/opt/trn_rl_repo" not in sys.path:
    sys.path.insert(0, "/opt/trn_rl_repo")

from contextlib import ExitStack

import ml_dtypes
import numpy as np

import concourse.bass as bass  # noqa: F401
import concourse.mybir as mybir
import concourse.tile as tile
from concourse import bacc, bass_utils

P, LMAX, D, H = 4096, 64, 512, 512
NCORES = 8
PS = P // NCORES          # paths per core
BP = 16                   # paths per superblock
NB = PS // BP             # superblocks per core
R = BP * LMAX             # rows per superblock
KC = D // 128             # contraction chunks
HC = H // 128             # hidden tiles
MASK_NEG = -30000.0

f32 = mybir.dt.float32
bf16 = mybir.dt.bfloat16
AF = mybir.ActivationFunctionType
ALU = mybir.AluOpType
AX = mybir.AxisListType

LAST_RESULT = None
_PROG = None
_TRACE_KW = {}


def _bcast_free(ap, n, axis_total):
    """[P, F] -> [P, n, F] with a step-0 broadcast middle dim."""
    return ap.rearrange("p (x r) -> p x r", x=1).to_broadcast(axis_total)


def _build_program(nb=NB, stage="full"):
    nc = bacc.Bacc("TRN2", target_bir_lowering=False, debug=False, num_devices=NCORES)

    xt = nc.dram_tensor("xt", [nb, KC, 128, R], bf16, kind="ExternalInput")
    msk = nc.dram_tensor("msk", [nb, R], bf16, kind="ExternalInput")
    w1 = nc.dram_tensor("w1", [KC, 128, H], bf16, kind="ExternalInput")
    w2 = nc.dram_tensor("w2", [128, HC], bf16, kind="ExternalInput")
    b1 = nc.dram_tensor("b1", [128, HC], f32, kind="ExternalInput")
    aw1 = nc.dram_tensor("aw1", [KC, 128, H], f32, kind="ExternalInput")
    ab1 = nc.dram_tensor("ab1", [128, HC], f32, kind="ExternalInput")
    aw2 = nc.dram_tensor("aw2", [128, HC], f32, kind="ExternalInput")
    one1_bf = nc.dram_tensor("one1_bf", [1, 1], bf16, kind="ExternalInput")
    out_part = nc.dram_tensor("out_part", [128, KC], f32, kind="ExternalOutput")
    out_stats = nc.dram_tensor("out_stats", [1, 2], f32, kind="ExternalOutput")
    ps_here = nb * BP
    dbg = None
    if stage != "full":
        dbg = nc.dram_tensor("dbg", [128, KC, ps_here], f32, kind="ExternalOutput")

    with ExitStack() as ctx:
        tc = ctx.enter_context(tile.TileContext(nc))
        const = ctx.enter_context(tc.tile_pool(name="const", bufs=1))
        xpool = ctx.enter_context(tc.tile_pool(name="x", bufs=3))
        xwpool = ctx.enter_context(tc.tile_pool(name="xw", bufs=2))
        hpool = ctx.enter_context(tc.tile_pool(name="h", bufs=2))
        wpool = ctx.enter_context(tc.tile_pool(name="w", bufs=2))
        spool = ctx.enter_context(tc.tile_pool(name="s", bufs=3))
        ph_pool = ctx.enter_context(tc.tile_pool(name="ph", bufs=6, space="PSUM"))
        ps_pool = ctx.enter_context(tc.tile_pool(name="ps", bufs=2, space="PSUM"))

        t_w1 = const.tile([128, KC, H], bf16)
        nc.sync.dma_start(t_w1[:], w1.ap().rearrange("k d h -> d k h"))
        t_w2 = const.tile([128, HC], bf16)
        nc.sync.dma_start(t_w2[:], w2.ap())
        t_b1 = const.tile([128, HC], f32)
        nc.sync.dma_start(t_b1[:], b1.ap())
        t_aw1 = const.tile([128, KC, H], f32)
        nc.sync.dma_start(t_aw1[:], aw1.ap().rearrange("k d h -> d k h"))
        t_ab1 = const.tile([128, HC], f32)
        nc.sync.dma_start(t_ab1[:], ab1.ap())
        t_aw2 = const.tile([128, HC], f32)
        nc.sync.dma_start(t_aw2[:], aw2.ap())
        t_one1 = const.tile([1, 1], bf16)
        nc.sync.dma_start(t_one1[:], one1_bf.ap())

        pfT = const.tile([128, KC, ps_here], f32)  # normalized path features

        for b in range(nb):
            x_all = xpool.tile([128, KC, R], bf16, tag="x")
            nc.sync.dma_start(x_all[:], xt.ap()[b].rearrange("k d r -> d k r"))
            mrow = spool.tile([1, R], bf16, tag="mrow")
            nc.sync.dma_start(mrow[:], msk.ap()[b : b + 1, :])

            rh_list = [
                hpool.tile([128, R], bf16, tag=f"rh{j}", name=f"rh{j}_{b}")
                for j in range(HC)
            ]
            erow = spool.tile([1, R], bf16, tag="erow")
            for hh in range(2):
                hs = slice(512 * hh, 512 * (hh + 1))
                for j in range(HC):
                    ph = ph_pool.tile([128, 512], f32, tag="h")
                    for k in range(KC):
                        nc.tensor.matmul(
                            ph[:],
                            t_w1[:, k, 128 * j : 128 * (j + 1)],
                            x_all[:, k, hs],
                            start=(k == 0),
                            stop=(k == KC - 1),
                        )
                    nc.scalar.activation(
                        rh_list[j][:, hs], ph[:], AF.Relu, bias=t_b1[:, j : j + 1]
                    )

                # scores + additive length mask, one PSUM accumulation group
                ps_s = ps_pool.tile([1, 512], f32, tag="s")
                nc.tensor.matmul(
                    ps_s[:], t_one1[:], mrow[:, hs], start=True, stop=False,
                    skip_group_check=True,
                )
                for j in range(HC):
                    nc.tensor.matmul(
                        ps_s[:], t_w2[:, j : j + 1], rh_list[j][:, hs],
                        start=False, stop=(j == HC - 1),
                        skip_group_check=True,
                    )
                nc.scalar.activation(erow[:, hs], ps_s[:], AF.Exp)

            wful = wpool.tile([128, R], bf16, tag="wf")
            nc.gpsimd.partition_broadcast(wful[:], erow[:])

            wsum = spool.tile([128, BP], f32, tag="wsum")
            nc.vector.reduce_sum(
                wsum[:], wful[:].rearrange("p (s l) -> p s l", l=LMAX), axis=AX.X
            )
            winv = spool.tile([128, BP], f32, tag="winv")
            nc.vector.reciprocal(winv[:], wsum[:])

            xw = xwpool.tile([128, KC, R], bf16, tag="xw")
            nc.vector.tensor_mul(
                xw[:], x_all[:], _bcast_free(wful[:], KC, [128, KC, R])
            )
            praw = spool.tile([128, KC, BP], f32, tag="praw")
            nc.vector.reduce_sum(
                praw[:],
                xw[:].rearrange("p k (s l) -> p k s l", l=LMAX),
                axis=AX.X,
            )
            nc.vector.tensor_mul(
                pfT[:, :, BP * b : BP * (b + 1)],
                praw[:],
                _bcast_free(winv[:], KC, [128, KC, BP]),
            )

        if stage == "wsum":
            nc.sync.dma_start(dbg.ap(), pfT[:])

        if stage == "full":
            # ---- stage 2: path-level attention (fp32) ----
            rh2_list = []
            for j in range(HC):
                ph2 = ph_pool.tile([128, ps_here], f32, tag="h")
                for k in range(KC):
                    nc.tensor.matmul(
                        ph2[:],
                        t_aw1[:, k, 128 * j : 128 * (j + 1)],
                        pfT[:, k, :],
                        start=(k == 0),
                        stop=(k == KC - 1),
                    )
                rh2 = hpool.tile([128, ps_here], f32, tag=f"rh2{j}")
                nc.scalar.activation(rh2[:], ph2[:], AF.Relu, bias=t_ab1[:, j : j + 1])
                rh2_list.append(rh2)

            ps_a = ps_pool.tile([1, ps_here], f32, tag="s")
            for j in range(HC):
                nc.tensor.matmul(
                    ps_a[:], t_aw2[:, j : j + 1], rh2_list[j][:],
                    start=(j == 0), stop=(j == HC - 1),
                )

            negm = spool.tile([1, 1], f32, tag="negm")
            nc.vector.reduce_max(negm[:], ps_a[:], axis=AX.X, negate=True)
            ea = spool.tile([1, ps_here], f32, tag="ea")
            s_t = spool.tile([1, 1], f32, tag="s1")
            nc.scalar.activation(ea[:], ps_a[:], AF.Exp, bias=negm[:], accum_out=s_t[:])

            ebc = wpool.tile([128, ps_here], f32, tag="ebc")
            nc.gpsimd.partition_broadcast(ebc[:], ea[:])

            part = spool.tile([128, KC], f32, tag="part")
            for k in range(KC):
                scr = wpool.tile([128, ps_here], f32, tag="scr")
                nc.vector.tensor_mul(scr[:], pfT[:, k, :], ebc[:])
                nc.vector.reduce_sum(part[:, k : k + 1], scr[:], axis=AX.X)
            nc.sync.dma_start(out_part.ap(), part[:])
            nc.sync.dma_start(out_stats.ap()[:, 0:1], negm[:])
            nc.sync.dma_start(out_stats.ap()[:, 1:2], s_t[:])

    nc.compile()
    return nc


def _get_program():
    global _PROG
    if _PROG is None:
        _PROG = _build_program()
    return _PROG


def kernel(**inputs):
    global LAST_RESULT
    x = np.asarray(inputs["paths_nodes"], dtype=np.float32)
    lengths = np.asarray(inputs["lengths"], dtype=np.int32)
    pW1 = np.asarray(inputs["pW1"], dtype=np.float32)
    pb1 = np.asarray(inputs["pb1"], dtype=np.float32)
    pw2 = np.asarray(inputs["pw2"], dtype=np.float32)
    aW1 = np.asarray(inputs["aW1"], dtype=np.float32)
    ab1 = np.asarray(inputs["ab1"], dtype=np.float32)
    aw2 = np.asarray(inputs["aw2"], dtype=np.float32)
    # pb2 / ab2 shift their softmax logits uniformly -> no effect on output.

    nc = _get_program()

    bf = ml_dtypes.bfloat16
    xt_all = (
        x.reshape(NCORES, NB, BP, LMAX, KC, 128)
        .transpose(0, 1, 4, 5, 2, 3)
        .astype(bf)
        .reshape(NCORES, NB, KC, 128, R)
    )
    valid = np.arange(LMAX)[None, :] < lengths[:, None]
    msk_all = (
        np.where(valid, 0.0, MASK_NEG)
        .astype(bf)
        .reshape(NCORES, NB, R)
    )

    w1_np = np.ascontiguousarray(pW1.reshape(KC, 128, H)).astype(bf)
    w2_np = np.ascontiguousarray(pw2.reshape(HC, 128).T).astype(bf)
    b1_np = np.ascontiguousarray(pb1.reshape(HC, 128).T).astype(np.float32)
    aw1_np = np.ascontiguousarray(aW1.reshape(KC, 128, H)).astype(np.float32)
    ab1_np = np.ascontiguousarray(ab1.reshape(HC, 128).T).astype(np.float32)
    aw2_np = np.ascontiguousarray(aw2.reshape(HC, 128).T).astype(np.float32)
    one1 = np.ones((1, 1), dtype=bf)

    in_maps = []
    for c in range(NCORES):
        in_maps.append(
            {
                "xt": xt_all[c],
                "msk": msk_all[c],
                "w1": w1_np,
                "w2": w2_np,
                "b1": b1_np,
                "aw1": aw1_np,
                "ab1": ab1_np,
                "aw2": aw2_np,
                "one1_bf": one1,
            }
        )

    res = bass_utils.run_bass_kernel_spmd(
        nc, in_maps, core_ids=list(range(NCORES)), **_TRACE_KW
    )
    LAST_RESULT = res

    parts = np.stack([r["out_part"] for r in res.results])    # [8, 128, KC]
    stats = np.stack([r["out_stats"] for r in res.results])   # [8, 1, 2]
    m = -stats[:, 0, 0]
    s = stats[:, 0, 1]
    mg = m.max()
    sc = np.exp(m - mg)
    total = float((sc * s).sum())
    vec = (parts * sc[:, None, None]).sum(axis=0)             # [128, KC]
    user = np.ascontiguousarray(vec.T).reshape(D) / total
    return user.astype(np.float32)


# revision 15
# speedup vs baseline: 1.5753x; 1.4953x over previous
"""Trainium2 Bass kernel for nn_AttentionNetwork (ragged path attention).

Data-parallel over 8 NeuronCores: 512 paths per core. Paths are sorted by
length (host-side) and grouped into blocks of 16 with a per-block row
capacity = max length in the block (same capacities on every core =
element-wise max, so one SPMD program serves all 8). This skips the
~50% of node slots beyond each path's length that a fixed 64-slot layout
would waste. Per block: node-MLP (bf16 matmuls, h^T layout), length-masked
softmax over nodes (additive mask folded into the score-matmul PSUM group
as a K=1 accumulate; exp row broadcast across partitions on GpSimd), then
the softmax-weighted node sum on the vector engine. Stage 2 (fp32)
computes path-attention scores and returns exp-weighted partial sums +
(max, sumexp) stats; the host combines the 8 partials (softmax over paths
is permutation-invariant, so the sorted order needs no undoing).
"""

import sys

if "/opt/trn_rl_repo" not in sys.path:
    sys.path.insert(0, "/opt/trn_rl_repo")

from contextlib import ExitStack

import ml_dtypes
import numpy as np

import concourse.bass as bass  # noqa: F401
import concourse.mybir as mybir
import concourse.tile as tile
from concourse import bacc, bass_utils

P, LMAX, D, H = 4096, 64, 512, 512
NCORES = 8
PS = P // NCORES          # paths per core
BP = 16                   # paths per block
NB = PS // BP             # blocks per core
KC = D // 128             # contraction chunks
HC = H // 128             # hidden tiles
MASK_NEG = -30000.0
MAXROWS = BP * LMAX       # row capacity upper bound per block

f32 = mybir.dt.float32
bf16 = mybir.dt.bfloat16
AF = mybir.ActivationFunctionType
ALU = mybir.AluOpType
AX = mybir.AxisListType

LAST_RESULT = None
_PROG_CACHE = {}
_TRACE_KW = {}


def _build_program(caps, stage="full"):
    """caps: tuple of NB per-block row-capacities-per-path (1..LMAX)."""
    nb = len(caps)
    rows_list = [BP * c for c in caps]
    tot_rows = sum(rows_list)
    ps_here = nb * BP

    nc = bacc.Bacc("TRN2", target_bir_lowering=False, debug=False, num_devices=NCORES)

    xt = nc.dram_tensor("xt", [KC * 128 * tot_rows], bf16, kind="ExternalInput")
    msk = nc.dram_tensor("msk", [tot_rows], bf16, kind="ExternalInput")
    w1 = nc.dram_tensor("w1", [KC, 128, H], bf16, kind="ExternalInput")
    w2 = nc.dram_tensor("w2", [128, HC], bf16, kind="ExternalInput")
    b1 = nc.dram_tensor("b1", [128, HC], f32, kind="ExternalInput")
    aw1 = nc.dram_tensor("aw1", [KC, 128, H], f32, kind="ExternalInput")
    ab1 = nc.dram_tensor("ab1", [128, HC], f32, kind="ExternalInput")
    aw2 = nc.dram_tensor("aw2", [128, HC], f32, kind="ExternalInput")
    one1_bf = nc.dram_tensor("one1_bf", [1, 1], bf16, kind="ExternalInput")
    out_part = nc.dram_tensor("out_part", [128, KC], f32, kind="ExternalOutput")
    out_stats = nc.dram_tensor("out_stats", [1, 2], f32, kind="ExternalOutput")
    dbg = None
    if stage != "full":
        dbg = nc.dram_tensor("dbg", [128, KC, ps_here], f32, kind="ExternalOutput")

    with ExitStack() as ctx:
        tc = ctx.enter_context(tile.TileContext(nc))
        const = ctx.enter_context(tc.tile_pool(name="const", bufs=1))
        xpool = ctx.enter_context(tc.tile_pool(name="x", bufs=3))
        xwpool = ctx.enter_context(tc.tile_pool(name="xw", bufs=2))
        hpool = ctx.enter_context(tc.tile_pool(name="h", bufs=2))
        wpool = ctx.enter_context(tc.tile_pool(name="w", bufs=2))
        spool = ctx.enter_context(tc.tile_pool(name="s", bufs=3))
        ph_pool = ctx.enter_context(tc.tile_pool(name="ph", bufs=6, space="PSUM"))
        ps_pool = ctx.enter_context(tc.tile_pool(name="ps", bufs=2, space="PSUM"))

        t_w1 = const.tile([128, KC, H], bf16)
        nc.sync.dma_start(t_w1[:], w1.ap().rearrange("k d h -> d k h"))
        t_w2 = const.tile([128, HC], bf16)
        nc.sync.dma_start(t_w2[:], w2.ap())
        t_b1 = const.tile([128, HC], f32)
        nc.sync.dma_start(t_b1[:], b1.ap())
        t_aw1 = const.tile([128, KC, H], f32)
        nc.sync.dma_start(t_aw1[:], aw1.ap().rearrange("k d h -> d k h"))
        t_ab1 = const.tile([128, HC], f32)
        nc.sync.dma_start(t_ab1[:], ab1.ap())
        t_aw2 = const.tile([128, HC], f32)
        nc.sync.dma_start(t_aw2[:], aw2.ap())
        t_one1 = const.tile([1, 1], bf16)
        nc.sync.dma_start(t_one1[:], one1_bf.ap())

        pfT = const.tile([128, KC, ps_here], f32)  # normalized path features

        x_off = 0
        m_off = 0
        for b in range(nb):
            cap = caps[b]
            rows = rows_list[b]
            x_all = xpool.tile([128, KC, rows], bf16, tag="x", name=f"x_{b}")
            nc.sync.dma_start(
                x_all[:],
                xt.ap()[x_off : x_off + KC * 128 * rows].rearrange(
                    "(k d r) -> d k r", k=KC, d=128
                ),
            )
            mrow = spool.tile([1, rows], bf16, tag="mrow", name=f"mrow_{b}")
            nc.sync.dma_start(
                mrow[:], msk.ap()[m_off : m_off + rows].rearrange("(o r) -> o r", o=1)
            )
            x_off += KC * 128 * rows
            m_off += rows

            rh_list = [
                hpool.tile([128, rows], bf16, tag=f"rh{j}", name=f"rh{j}_{b}")
                for j in range(HC)
            ]
            erow = spool.tile([1, rows], bf16, tag="erow", name=f"erow_{b}")
            chunks = []
            off = 0
            while off < rows:
                sz = min(512, rows - off)
                chunks.append((off, sz))
                off += sz
            for (coff, csz) in chunks:
                cs = slice(coff, coff + csz)
                for j in range(HC):
                    ph = ph_pool.tile([128, csz], f32, tag="h", name=f"ph{j}_{b}_{coff}")
                    for k in range(KC):
                        nc.tensor.matmul(
                            ph[:],
                            t_w1[:, k, 128 * j : 128 * (j + 1)],
                            x_all[:, k, cs],
                            start=(k == 0),
                            stop=(k == KC - 1),
                        )
                    nc.scalar.activation(
                        rh_list[j][:, cs], ph[:], AF.Relu, bias=t_b1[:, j : j + 1]
                    )

                # scores + additive length mask, one PSUM accumulation group
                ps_s = ps_pool.tile([1, csz], f32, tag="s", name=f"ps_{b}_{coff}")
                nc.tensor.matmul(
                    ps_s[:], t_one1[:], mrow[:, cs], start=True, stop=False,
                    skip_group_check=True,
                )
                for j in range(HC):
                    nc.tensor.matmul(
                        ps_s[:], t_w2[:, j : j + 1], rh_list[j][:, cs],
                        start=False, stop=(j == HC - 1),
                        skip_group_check=True,
                    )
                nc.scalar.activation(erow[:, cs], ps_s[:], AF.Exp)

            wful = wpool.tile([128, rows], bf16, tag="wf", name=f"wf_{b}")
            nc.gpsimd.partition_broadcast(wful[:], erow[:])

            wsum = spool.tile([128, BP], f32, tag="wsum", name=f"wsum_{b}")
            nc.vector.reduce_sum(
                wsum[:], wful[:].rearrange("p (s l) -> p s l", l=cap), axis=AX.X
            )
            winv = spool.tile([128, BP], f32, tag="winv", name=f"winv_{b}")
            nc.vector.reciprocal(winv[:], wsum[:])

            xw = xwpool.tile([128, KC, rows], bf16, tag="xw", name=f"xw_{b}")
            for k in range(KC):
                nc.vector.tensor_mul(xw[:, k, :], x_all[:, k, :], wful[:])
            praw = spool.tile([128, KC, BP], f32, tag="praw", name=f"praw_{b}")
            nc.vector.reduce_sum(
                praw[:],
                xw[:].rearrange("p k (s l) -> p k s l", l=cap),
                axis=AX.X,
            )
            winv_bc = winv[:].rearrange("p (x s) -> p x s", x=1).to_broadcast(
                [128, KC, BP]
            )
            nc.vector.tensor_mul(
                pfT[:, :, BP * b : BP * (b + 1)], praw[:], winv_bc
            )

        if stage == "wsum":
            nc.sync.dma_start(dbg.ap(), pfT[:])

        if stage == "full":
            # ---- stage 2: path-level attention (fp32) ----
            rh2_list = []
            for j in range(HC):
                ph2 = ph_pool.tile([128, ps_here], f32, tag="h")
                for k in range(KC):
                    nc.tensor.matmul(
                        ph2[:],
                        t_aw1[:, k, 128 * j : 128 * (j + 1)],
                        pfT[:, k, :],
                        start=(k == 0),
                        stop=(k == KC - 1),
                    )
                rh2 = hpool.tile([128, ps_here], f32, tag=f"rh2{j}")
                nc.scalar.activation(rh2[:], ph2[:], AF.Relu, bias=t_ab1[:, j : j + 1])
                rh2_list.append(rh2)

            ps_a = ps_pool.tile([1, ps_here], f32, tag="s")
            for j in range(HC):
                nc.tensor.matmul(
                    ps_a[:], t_aw2[:, j : j + 1], rh2_list[j][:],
                    start=(j == 0), stop=(j == HC - 1),
                )

            negm = spool.tile([1, 1], f32, tag="negm")
            nc.vector.reduce_max(negm[:], ps_a[:], axis=AX.X, negate=True)
            ea = spool.tile([1, ps_here], f32, tag="ea")
            s_t = spool.tile([1, 1], f32, tag="s1")
            nc.scalar.activation(ea[:], ps_a[:], AF.Exp, bias=negm[:], accum_out=s_t[:])

            ebc = wpool.tile([128, ps_here], f32, tag="ebc")
            nc.gpsimd.partition_broadcast(ebc[:], ea[:])

            part = spool.tile([128, KC], f32, tag="part")
            for k in range(KC):
                scr = wpool.tile([128, ps_here], f32, tag="scr", name=f"scr_{k}")
                nc.vector.tensor_mul(scr[:], pfT[:, k, :], ebc[:])
                nc.vector.reduce_sum(part[:, k : k + 1], scr[:], axis=AX.X)
            nc.sync.dma_start(out_part.ap(), part[:])
            nc.sync.dma_start(out_stats.ap()[:, 0:1], negm[:])
            nc.sync.dma_start(out_stats.ap()[:, 1:2], s_t[:])

    nc.compile()
    return nc


def _get_program(caps, stage="full"):
    key = (tuple(caps), stage)
    if key not in _PROG_CACHE:
        _PROG_CACHE[key] = _build_program(tuple(caps), stage)
    return _PROG_CACHE[key]


def _prep(inputs):
    """Host-side sharding/sorting/packing. Returns (caps, in_maps)."""
    x = np.asarray(inputs["paths_nodes"], dtype=np.float32)
    lengths = np.asarray(inputs["lengths"], dtype=np.int32)
    pW1 = np.asarray(inputs["pW1"], dtype=np.float32)
    pb1 = np.asarray(inputs["pb1"], dtype=np.float32)
    pw2 = np.asarray(inputs["pw2"], dtype=np.float32)
    aW1 = np.asarray(inputs["aW1"], dtype=np.float32)
    ab1 = np.asarray(inputs["ab1"], dtype=np.float32)
    aw2 = np.asarray(inputs["aw2"], dtype=np.float32)
    # pb2 / ab2 shift their softmax logits uniformly -> no effect on output.

    bf = ml_dtypes.bfloat16
    len_sh = lengths.reshape(NCORES, PS)
    orders = np.argsort(-len_sh, axis=1, kind="stable")        # [NC, PS] desc
    sorted_len = np.take_along_axis(len_sh, orders, axis=1)
    # per-block capacity = max over cores of the block's max length
    caps = sorted_len.reshape(NCORES, NB, BP).max(axis=2).max(axis=0)  # [NB]
    caps = tuple(int(c) for c in caps)

    x_sh = x.reshape(NCORES, PS, LMAX, D)
    w1_np = np.ascontiguousarray(pW1.reshape(KC, 128, H)).astype(bf)
    w2_np = np.ascontiguousarray(pw2.reshape(HC, 128).T).astype(bf)
    b1_np = np.ascontiguousarray(pb1.reshape(HC, 128).T).astype(np.float32)
    aw1_np = np.ascontiguousarray(aW1.reshape(KC, 128, H)).astype(np.float32)
    ab1_np = np.ascontiguousarray(ab1.reshape(HC, 128).T).astype(np.float32)
    aw2_np = np.ascontiguousarray(aw2.reshape(HC, 128).T).astype(np.float32)
    one1 = np.ones((1, 1), dtype=bf)

    ar = np.arange(LMAX)
    in_maps = []
    for c in range(NCORES):
        xc = x_sh[c][orders[c]]                       # [PS, LMAX, D] sorted
        lc = sorted_len[c]                            # [PS]
        xt_parts = []
        mk_parts = []
        for b in range(NB):
            cap = caps[b]
            xb = xc[BP * b : BP * (b + 1), :cap, :]   # [BP, cap, D]
            # -> [KC, 128, BP*cap] with d on the middle axis
            xb_t = (
                xb.reshape(BP, cap, KC, 128)
                .transpose(2, 3, 0, 1)
                .reshape(KC, 128, BP * cap)
            )
            xt_parts.append(xb_t.astype(bf).ravel())
            lb = lc[BP * b : BP * (b + 1)]
            mk = np.where(ar[None, :cap] < lb[:, None], 0.0, MASK_NEG)
            mk_parts.append(mk.astype(bf).ravel())
        in_maps.append(
            {
                "xt": np.concatenate(xt_parts),
                "msk": np.concatenate(mk_parts),
                "w1": w1_np,
                "w2": w2_np,
                "b1": b1_np,
                "aw1": aw1_np,
                "ab1": ab1_np,
                "aw2": aw2_np,
                "one1_bf": one1,
            }
        )
    return caps, in_maps


def kernel(**inputs):
    global LAST_RESULT
    caps, in_maps = _prep(inputs)
    nc = _get_program(caps)

    res = bass_utils.run_bass_kernel_spmd(
        nc, in_maps, core_ids=list(range(NCORES)), **_TRACE_KW
    )
    LAST_RESULT = res

    parts = np.stack([r["out_part"] for r in res.results])    # [8, 128, KC]
    stats = np.stack([r["out_stats"] for r in res.results])   # [8, 1, 2]
    m = -stats[:, 0, 0]
    s = stats[:, 0, 1]
    mg = m.max()
    sc = np.exp(m - mg)
    total = float((sc * s).sum())
    vec = (parts * sc[:, None, None]).sum(axis=0)             # [128, KC]
    user = np.ascontiguousarray(vec.T).reshape(D) / total
    return user.astype(np.float32)
